# revision 22
# baseline (speedup 1.0000x reference)
"""LocallyConnected2d Bass kernel for 8 TRN2 NeuronCores.

Problem: out[b,o,oh,ow] = sum_{c,kh,kw} x[b,c,oh+kh-1,ow+kw-1] * w[o,c,oh,ow,kh*3+kw]
Shapes: x (8,64,32,32) f32, weight (1,64,64,32,32,9) f32 -> out (8,64,32,32) f32.

Sharding: each core owns 4 consecutive output rows (oh). The 144 MiB weight
tensor is the dominant traffic; this split reads it exactly once (9.4 MB/core
in bf16) with no duplication and needs no collectives.

Per-core kernel: every output location is an independent tiny matmul
  out_loc[b, o] = patches_loc[ck, b].T @ w_loc[ck, o]
PSUM-accumulated over tap groups (M=b=8, N=o=64). The 9 taps are packed
into 5 K=128 matmuls per location: four "tap pair" matmuls whose 128
contraction partitions hold (tapA c | tapB c) — partitions 64-127 of the x
tiles are pre-shifted copies so one base offset addresses both taps (pairs
(0,1)(3,4)(6,7) via a column-shifted copy, (2,5) via a row-shifted copy) —
plus one tap-8 matmul against a zero-padded weight column (the unused
64-partition half is zeros in SBUF, set once by memset and never DMA'd).
Pair slots 0-1 (taps 0,1,3,4) are quantized to fp8-e4m3, the rest stay
bf16: measured 1.56e-2 max rel err vs the 2e-2 gate, for 25% less weight
traffic; per-matmul operand-dtype switching costs nothing on the PE.

PE scheduling notes (HW-measured):
- All matmuls are K=128: mixing K=128/K=64 tile sizes in an accumulation
  stream costs a ~170ns PE reconfig per switch (this is why tap-8 is
  zero-padded to K=128 instead of run as a K=64 half-array matmul).
- Adjacent even/odd locations' accumulation chains are interleaved across
  two different PSUM banks, which keeps the PE at its ~35-38ns/matmul
  issue floor. Within a bank, groups stay sequential and contiguous:
  start=True clears the has_written bits of the WHOLE bank, so only one
  chain may be open per bank at a time.
- Both operands are laid out so the streamed dimension is contiguous per
  partition (x tiles carry B innermost, weight tiles carry O innermost) —
  strided PE operand reads cost ~8x (16B SBUF line per 2B element).

DMA notes: all input loads ride ONE Sync-HWDGE queue, issued in exact
consumption order (x, then row-0 weights/tap-8, then the remaining row
chunks) — SDMA engines alternate between queue rings at ~139KB packet
granularity, so a second queue's small transfers would starve behind the
big chunk packets. Everything is prefetched up front (all buffers
resident) so the 16 engines stream at the ~390 GB/s aggregate cap.
Output staging is bf16 (host converts back to f32), halving the PSUM
evacuation and store cost; out DMAs ride the Scalar queue.
"""

import numpy as np
import ml_dtypes

import concourse.bacc as bacc
import concourse.bass as bass
import concourse.tile as tile
from concourse import mybir
from concourse.bass_utils import run_bass_kernel_spmd

B, C, O = 8, 64, 64
OH, OW = 32, 32
NCORES = 8
R = OH // NCORES          # 4 oh rows per core
HS = R + 2                # x halo rows per core
WS = OW + 2               # padded width
F32 = mybir.dt.float32
NCHUNK = 2 * R            # main weight chunks per core (half an oh row each)

# Tap pairing: slots 0-3 are (tapA, tapB) pairs for even ow, 4-7 for odd;
# taps are k = 3*kh + kw.
PAIRS = [(0, 1), (3, 4), (6, 7), (2, 5)]
# lhsT base (kh, kw, which x tile) per pair slot; x tile 0 = column-shifted
# duplicate in partitions 64+, tile 1 = row-shifted duplicate.
PAIR_BASE = [(0, 0, 0), (1, 0, 0), (2, 0, 0), (0, 2, 1)]

USE_BF16 = True
DT = mybir.dt.bfloat16 if USE_BF16 else F32
NPDT = ml_dtypes.bfloat16 if USE_BF16 else np.float32
F8 = mybir.dt.float8e4
NPF8 = ml_dtypes.float8_e4m3

_cache: dict = {}
_last_in_maps = None


def _build() -> bass.Bass:
    nc = bacc.Bacc("TRN2", target_bir_lowering=False, debug=False,
                   num_devices=NCORES)
    # x patches, B innermost (contiguous lhsT): [0:64] = slab [c,h,w,b];
    # [64:128] = shifted duplicate.
    xa = nc.dram_tensor("xa", [128, HS, WS, B], DT, kind="ExternalInput").ap()
    xb = nc.dram_tensor("xb", [128, HS, WS, B], DT, kind="ExternalInput").ap()
    # pair weights, hybrid precision: slots 0-1 (taps 0,1,3,4) in fp8-e4m3,
    # slots 2-3 (taps 6,7,2,5) in bf16 — measured 1.6% rel err vs the 2%
    # gate, and 25% less weight traffic. Layout [oh_l, p, blk, j, eo, s, o],
    # contiguous per row chunk and O innermost (contiguous rhs).
    wf8 = nc.dram_tensor("wf8", [R, 128, 2, 8, 2, 2, O], F8,
                         kind="ExternalInput").ap()
    wbf = nc.dram_tensor("wbf", [R, 128, 2, 8, 2, 2, O], DT,
                         kind="ExternalInput").ap()
    # tap-8 weights, data halves only (c-major): even locs use partitions
    # 0-63, odd locs 64-127; the complementary halves stay zero in SBUF.
    t8e = nc.dram_tensor("t8e", [64, R, OW // 2, O], DT,
                         kind="ExternalInput").ap()
    t8o = nc.dram_tensor("t8o", [64, R, OW // 2, O], DT,
                         kind="ExternalInput").ap()
    out = nc.dram_tensor("out", [B, R, OW, O], DT, kind="ExternalOutput").ap()

    with tile.TileContext(nc) as tc:
        with (
            tc.tile_pool(name="xpool", bufs=1) as xpool,
            tc.tile_pool(name="wpool", bufs=NCHUNK) as wpool,
            tc.tile_pool(name="w8pool", bufs=1) as w8pool,
            tc.tile_pool(name="opool", bufs=1) as opool,
            tc.tile_pool(name="pspool", bufs=8, space="PSUM") as pspool,
        ):
            # All input loads ride the Sync queue in consumption order —
            # a second queue's small transfers get starved behind the big
            # chunk packets (engines alternate rings at ~139KB packet
            # granularity), so FIFO order on one ring is faster.
            x_sb = [xpool.tile([128, HS, WS, B], DT, name="xa_sb"),
                    xpool.tile([128, HS, WS, B], DT, name="xb_sb")]
            w8_sb = w8pool.tile([128, R, 2, OW // 2, O], DT, name="w8_sb")
            f8_sb = [wpool.tile([128, 2, 8, 2, 2, O], F8, tag="wf8",
                                name=f"wf8_{r}", bufs=R) for r in range(R)]
            bf_sb = [wpool.tile([128, 2, 8, 2, 2, O], DT, tag="wbf",
                                name=f"wbf_{r}", bufs=R) for r in range(R)]

            # tap-8 zero halves (disjoint from the DMA'd halves, so the
            # DMAs don't wait on them), split per row so row 0 is ready
            # before the first tap-8 matmul.
            for r in range(R):
                nc.vector.memset(w8_sb[64:128, r, 0], 0.0)
                nc.vector.memset(w8_sb[0:64, r, 1], 0.0)

            # One FIFO queue, issued in exact consumption order: row-0
            # weights, then x, then row-0 tap-8, then the rest of the
            # stream. All buffers stay resident so the queue drains
            # back-to-back at full rate.
            nc.sync.dma_start(x_sb[0][:], xa)
            nc.sync.dma_start(x_sb[1][:], xb)
            nc.sync.dma_start(f8_sb[0][:, 0], wf8[0, :, 0])
            nc.sync.dma_start(bf_sb[0][:, 0], wbf[0, :, 0])
            nc.sync.dma_start(w8_sb[0:64, 0, 0], t8e[:, 0])
            nc.sync.dma_start(w8_sb[64:128, 0, 1], t8o[:, 0])
            nc.sync.dma_start(f8_sb[0][:, 1], wf8[0, :, 1])
            nc.sync.dma_start(bf_sb[0][:, 1], wbf[0, :, 1])
            nc.sync.dma_start(w8_sb[0:64, 1:R, 0], t8e[:, 1:R])
            nc.sync.dma_start(w8_sb[64:128, 1:R, 1], t8o[:, 1:R])
            for r in range(1, R):
                # half-row granularity: same byte order, but completion
                # sems (incl. the straggler engine's last increment) fire
                # per half-row, so the PE's boundary waits cover less data
                for hb in range(2):
                    nc.sync.dma_start(f8_sb[r][:, hb], wf8[r, :, hb])
                    nc.sync.dma_start(bf_sb[r][:, hb], wbf[r, :, hb])

            out_sb = opool.tile([B, R, OW, O], DT)

            for oh_l in range(R):
                for blk in range(2):
                    # 4-way chain interleave across 4 PSUM banks (locations
                    # ow..ow+3 in flight); full-bank tiles keep each chain's
                    # bank private, groups within a bank stay contiguous.
                    pts = [pspool.tile([B, 8, O], F32, tag="ps",
                                       name=f"ps{c}_{oh_l}_{blk}")
                           for c in range(4)]
                    for q in range(4):
                        owA = 16 * blk + 4 * q
                        for s in range(4):
                            kh, kw, xt = PAIR_BASE[s]
                            for c in range(4):
                                j, eo = 2 * q + c // 2, c % 2
                                if s < 2:
                                    rhs = f8_sb[oh_l][:, blk, j, eo, s, :]
                                else:
                                    rhs = bf_sb[oh_l][:, blk, j, eo, s - 2, :]
                                nc.tensor.matmul(
                                    pts[c][:, q, :],
                                    x_sb[xt][:, oh_l + kh, owA + c + kw, :],
                                    rhs, start=(s == 0), stop=False)
                        # tap 8, K=128 against zero-padded weight columns;
                        # lhsT for chains c and c+1 resolve to one x column.
                        for c in range(4):
                            j, eo = 2 * q + c // 2, c % 2
                            nc.tensor.matmul(
                                pts[c][:, q, :],
                                x_sb[0][:, oh_l + 2, owA + 2 * (c // 2) + 2, :],
                                w8_sb[:, oh_l, eo, 8 * blk + j, :],
                                start=False, stop=True)
                    for c in range(4):
                        nc.vector.tensor_copy(
                            out=out_sb[:, oh_l, 16 * blk + c:16 * blk + 16:4, :],
                            in_=pts[c][:, 0:4, :])
                if oh_l % 2 == 1:
                    nc.scalar.dma_start(out[:, oh_l - 1:oh_l + 1],
                                        out_sb[:, oh_l - 1:oh_l + 1])
    nc.compile()
    return nc


def _marshal(x: np.ndarray, weight: np.ndarray) -> list[dict]:
    x = np.ascontiguousarray(x, dtype=np.float32)
    w = weight[0]  # (O, C, OH, OW, K)

    # Padded input (B, C, OH+2, OW+2); core r reads padded rows [R*r, R*r+HS)
    xp = np.zeros((B, C, OH + 2, OW + 2), dtype=np.float32)
    xp[:, :, 1:OH + 1, 1:OW + 1] = x

    in_maps = []
    for r in range(NCORES):
        # [c, h, w, b] with b innermost so lhsT reads are contiguous
        slab = xp[:, :, R * r:R * r + HS, :].transpose(1, 2, 3, 0)
        sw = np.zeros_like(slab)
        sw[:, :, :WS - 1, :] = slab[:, :, 1:, :]        # column shift
        sh = np.zeros_like(slab)
        sh[:, :HS - 1, :, :] = slab[:, 1:, :, :]        # row shift
        xa_r = np.concatenate([slab, sw], axis=0).astype(NPDT)
        xb_r = np.concatenate([slab, sh], axis=0).astype(NPDT)

        # weight slab; wt: [oh, c, o, ow, k]
        wt = w[:, :, R * r:R * (r + 1), :, :].transpose(2, 1, 0, 3, 4)
        even, odd = wt[:, :, :, 0::2, :], wt[:, :, :, 1::2, :]
        W2 = np.empty((R, 128, 2, 4, O, OW // 2), dtype=np.float32)
        for s, (ka, kb) in enumerate(PAIRS):
            W2[:, 0:64, 0, s] = even[..., ka]
            W2[:, 64:128, 0, s] = even[..., kb]
            W2[:, 0:64, 1, s] = odd[..., ka]
            W2[:, 64:128, 1, s] = odd[..., kb]
        # [oh, p, eo, s, o, owp] -> [oh, p, blk, j, eo, s, o]
        W4 = W2.reshape(R, 128, 2, 4, O, 2, 8).transpose(0, 1, 5, 6, 2, 3, 4)
        wf8_r = W4[..., 0:2, :]
        wbf_r = W4[..., 2:4, :]
        # tap-8 data halves: [c, oh, m, o]
        t8e_r = even[..., 8].transpose(1, 0, 3, 2)
        t8o_r = odd[..., 8].transpose(1, 0, 3, 2)
        in_maps.append({
            "xa": np.ascontiguousarray(xa_r),
            "xb": np.ascontiguousarray(xb_r),
            "wf8": np.ascontiguousarray(wf8_r.astype(NPF8)),
            "wbf": np.ascontiguousarray(wbf_r.astype(NPDT)),
            "t8e": np.ascontiguousarray(t8e_r.astype(NPDT)),
            "t8o": np.ascontiguousarray(t8o_r.astype(NPDT)),
        })
    return in_maps


def kernel(x: np.ndarray, weight: np.ndarray) -> np.ndarray:
    global _last_in_maps
    in_maps = _marshal(x, weight)
    _last_in_maps = in_maps

    if "nc" not in _cache:
        _cache["nc"] = _build()
    res = run_bass_kernel_spmd(_cache["nc"], in_maps, list(range(NCORES)))

    # Per-core out is (B, R, OW, O); stitch to (B, O, OH, OW).
    parts = [res.results[r]["out"].astype(np.float32).transpose(0, 3, 1, 2)
             for r in range(NCORES)]
    return np.ascontiguousarray(np.concatenate(parts, axis=2))
